# revision 1
# baseline (speedup 1.0000x reference)
"""GQA attention kernel for 8 Trainium2 NeuronCores.

Sharding: core = (batch b, kv_group g), b in {0,1}, g in {0..3}.
Each core computes the 4 heads of one KV group for one batch and the
partial output projection for those heads; the host sums the 4 group
partials per batch.  Zero duplicated compute across cores.

All matmul operands are bf16 (fp32 PSUM accumulation); verified to give
~6e-3 max rel err vs the fp32 reference (tolerance 2e-2).

Performance structure (the tensor engine is the bottleneck; the whole
kernel keeps its queue dense and stall-free — ~87% MFU):
  - phase 1 (QKV projections): 6 concurrent PSUM accumulation groups;
    sc0 runs all Q matmuls first since wq/x arrive on the DMA queues
    before wk/wv.  x-tiles stream on the SP DMA queue, weights on the
    ACT-engine DMA queue (DMA instructions block their issuing engine
    for the whole transfer, so placement matters), V is transposed by
    the DMA xbar.  wk/wv are host-prearranged partition-major; x/wq
    keep strided patterns — an all-contiguous layout was measured to
    trigger a chip-wide ~20% DVFS clock drop.
  - phase 2 (attention): one flat software-pipelined stream across all
    (head-pair, kv-tile) steps: per step 2 scores matmuls into one
    2-bank PSUM tile, ONE exp (halves ACT per-op overhead; ACT is
    ~86% busy), and the sums/AV matmuls of the step TWO back, so exp
    latency hides behind ~8 matmuls and pair boundaries are seamless.
  - softmax: denominators via ones-stationary matmul (partition
    reduction on PE); PSUM banks freed by fast DVE/ACT copies in
    bank-reuse order; reciprocal + normalize run off-critical-path.
    The last pair's normalize is deferred into phase 3 (writing a
    separate tile — outT subtile deps would stall all of phase 3) and
    its sc<=1 region routes PSUM drains to ACT / outputs to sync so
    the deferred DVE chain never blocks PE.
  - phase 3 (output projection): transposed accumulation overlapping
    phase 2's drain tail; bf16 partial outputs on both DMA queues.
"""

import numpy as np

# problem shape (hardcoded per contract)
B, S, E = 2, 2048, 2048
H, G, D = 16, 4, 128
R = H // G          # heads per kv group = 4
ST = S // 128       # 16 t-tiles
ET = E // 128       # 16 e-tiles
SC = S // 512       # 4 s-chunks
QC = S // 512       # 4 q-chunks
NO = R + 2          # projection outputs per e-tile: 4x Q slices, K, V

_cache = {}


def _split_multi_waits(nc, maxw=1):
    """Walrus in this container accepts only one sync-wait per
    instruction; move extra waits onto preceding same-engine NoOps."""
    from concourse import mybir

    n_split = 0
    for fn in nc.m.functions:
        for bb in fn.blocks:
            out = []
            changed = False
            for inst in bb.instructions:
                si = inst.sync_info
                waits = list(si.on_wait or []) if si is not None else []
                if len(waits) > maxw:
                    changed = True
                    n_split += 1
                    head, tail = waits[:-maxw], waits[-maxw:]
                    for j in range(0, len(head), maxw):
                        nop = mybir.InstNoOp(
                            name=f"{inst.name}-wsplit{j}", ins=[], outs=[]
                        )
                        nop.engine = inst.engine
                        nop.sync_info = mybir.SyncInfo(
                            on_wait=head[j : j + maxw], on_update=[]
                        )
                        out.append(nop)
                    si.on_wait = tail
                out.append(inst)
            if changed:
                bb.instructions = out
    return n_split


def _build_program():
    import contextlib

    import concourse.bass as bass
    import concourse.tile as tile
    from concourse import mybir

    BF16 = mybir.dt.bfloat16
    F32 = mybir.dt.float32
    Exp = mybir.ActivationFunctionType.Exp
    Mult = mybir.AluOpType.mult

    nc = bass.Bass(target_bir_lowering=False)

    xT = nc.dram_tensor("xT", [E, S], BF16, kind="ExternalInput")
    wq = nc.dram_tensor("wq", [E, R * D], BF16, kind="ExternalInput")
    # wk/wv host-prearranged partition-major: their natural layout would
    # DMA as 256B rows at a fraction of peak
    wkh = nc.dram_tensor("wkh", [128, ET, D], BF16, kind="ExternalInput")
    wvh = nc.dram_tensor("wvh", [128, ET, D], BF16, kind="ExternalInput")
    wo = nc.dram_tensor("wo", [R * D, E], BF16, kind="ExternalInput")
    bqv = nc.dram_tensor("bqv", [R * D], F32, kind="ExternalInput")
    bkv = nc.dram_tensor("bkv", [D], F32, kind="ExternalInput")
    bvv = nc.dram_tensor("bvv", [D], F32, kind="ExternalInput")
    otd = nc.dram_tensor("ot", [E, S], BF16, kind="ExternalOutput")

    with tile.TileContext(nc) as tc:
        with contextlib.ExitStack() as ctx:
            consts = ctx.enter_context(tc.tile_pool(name="consts", bufs=1))
            big = ctx.enter_context(tc.tile_pool(name="big", bufs=1))

            bq_sb = consts.tile([128, R], F32)
            nc.gpsimd.dma_start(bq_sb, bqv.rearrange("(o p) -> p o", p=128))
            bk_sb = consts.tile([128, 1], F32)
            nc.gpsimd.dma_start(bk_sb, bkv.rearrange("(o p) -> p o", p=128))
            bv_sb = consts.tile([128, 1], F32)
            nc.gpsimd.dma_start(bv_sb, bvv.rearrange("(o p) -> p o", p=128))

            ones_f = consts.tile([128, 128], F32)
            nc.gpsimd.memset(ones_f, 1.0)
            ones = consts.tile([128, 128], BF16)
            nc.vector.tensor_copy(ones, ones_f)

            QT = big.tile([128, R, S], BF16)    # QT[d, h, q]
            KT = big.tile([128, S], BF16)       # KT[d, t]
            VT = big.tile([128, S], BF16)       # VT[d, t]
            V = big.tile([128, ST, D], BF16)    # V[t%128, tt, d]
            outT = big.tile([128, R, S], BF16)  # normalized attn out
            # last pair's normalized output lands here so phase-3's early
            # groups don't inherit a dependency on the deferred normalize
            outT_last = big.tile([128, 2, 512], BF16)
            wo_sb = big.tile([128, R, E], BF16)

            # ---- phase 1: QKV^T projections ----
            with tc.tile_pool(name="wts", bufs=1) as wpool, \
                 tc.tile_pool(name="xts", bufs=2) as xtpool, \
                 tc.tile_pool(name="ps1", bufs=8, space="PSUM") as ps1:
                wq_sb = wpool.tile([128, ET, R * D], BF16)
                wk_sb = wpool.tile([128, ET, D], BF16)
                wv_sb = wpool.tile([128, ET, D], BF16)
                # weights ride the ACT-engine DMA queue in multi-e-tile
                # chunks (per-DMA overhead ~0.6us dominates small transfers);
                # x-tiles ride the SP queue, also chunked
                xt0 = xtpool.tile([128, ET, 512], BF16, tag="xt")

                def _wchunk(dst, src_t, e0, e1):
                    nc.scalar.dma_start(
                        dst[:, e0:e1],
                        src_t[e0 * 128 : e1 * 128, :].rearrange(
                            "(o p) m -> p o m", p=128
                        ),
                    )

                _wchunk(wq_sb, wq, 0, 2)
                _wchunk(wq_sb, wq, 2, 6)
                nc.scalar.dma_start(wk_sb, wkh[:, :])
                _wchunk(wq_sb, wq, 6, 10)
                nc.scalar.dma_start(wv_sb, wvh[:, :])
                _wchunk(wq_sb, wq, 10, 14)
                _wchunk(wq_sb, wq, 14, 16)
                for k in range(4):
                    nc.sync.dma_start(
                        xt0[:, k * 4 : (k + 1) * 4],
                        xT[k * 512 : (k + 1) * 512, 0:512].rearrange(
                            "(o p) m -> p o m", p=128
                        ),
                    )

                for sc in range(SC):
                    if sc == 0:
                        xtile = xt0
                    else:
                        xtile = xtpool.tile([128, ET, 512], BF16, tag="xt")
                        for k in range(4):
                            nc.sync.dma_start(
                                xtile[:, k * 4 : (k + 1) * 4],
                                xT[k * 512 : (k + 1) * 512,
                                   sc * 512 : (sc + 1) * 512].rearrange(
                                    "(o p) m -> p o m", p=128
                                ),
                            )
                    cs = slice(sc * 512, (sc + 1) * 512)
                    pss = [ps1.tile([128, 512], F32, tag="p1",
                                    name=f"p1_{sc}_{i}")
                           for i in range(NO)]
                    # sc0 runs all Q matmuls first: wq/x arrive on the
                    # queues before wk/wv, so compute starts ~3us earlier
                    if sc == 0:
                        ot_passes = [(0, 1, 2, 3), (R, R + 1)]
                    else:
                        # pass-major: each group stops (and drains) as early
                        # as possible, so no drain burst at the sc boundary
                        ot_passes = [(R,), (R + 1,), (0,), (1,), (2,), (3,)]
                    def _drain(ot):
                        if ot < R:
                            dst, b = QT[:, ot, cs], bq_sb[:, ot : ot + 1]
                        elif ot == R:
                            dst, b = KT[:, cs], bk_sb[:, 0:1]
                        else:
                            dst, b = VT[:, cs], bv_sb[:, 0:1]
                        if ot % 2 == 0:
                            nc.scalar.add(dst, pss[ot], b)
                        else:
                            nc.vector.tensor_scalar_add(dst, pss[ot], b)

                    for ots in ot_passes:
                        for e in range(ET):
                            for ot in ots:
                                if ot < R:
                                    lhsT = wq_sb[:, e,
                                                 ot * 128 : (ot + 1) * 128]
                                elif ot == R:
                                    lhsT = wk_sb[:, e]
                                else:
                                    lhsT = wv_sb[:, e]
                                nc.tensor.matmul(
                                    pss[ot], lhsT, xtile[:, e],
                                    start=(e == 0), stop=(e == ET - 1),
                                )
                        if len(ots) == 1:
                            _drain(ots[0])
                    if len(ot_passes[0]) > 1:
                        for ot in range(NO):
                            _drain(ot)
                    # V transpose for this chunk's 4 t-tiles via the DMA xbar
                    for tt in range(sc * 4, sc * 4 + 4):
                        nc.sync.dma_start_transpose(
                            V[:, tt], VT[:, tt * 128 : (tt + 1) * 128]
                        )

                # wo is needed only in phase 3; SP queue is idle by then
                nc.sync.dma_start(wo_sb, wo.rearrange("(o p) m -> p o m", p=128))

            # ---- phase 2: attention as one continuous stream ----
            # Steps s: scores+exp for stream-pair step s, consume (sums/AV)
            # matmuls for step s-2.  Uniform 6 matmuls per step, across pair
            # boundaries too, so the PE never outruns exp or PSUM rotation.
            ppool = ctx.enter_context(tc.tile_pool(name="probs", bufs=4))
            spool = ctx.enter_context(tc.tile_pool(name="ssb", bufs=4))
            avspool = ctx.enter_context(tc.tile_pool(name="avsb", bufs=4))
            rpool = ctx.enter_context(tc.tile_pool(name="rcs", bufs=4))
            with tc.tile_pool(name="ps_sc", bufs=2, space="PSUM") as scpool, \
                 tc.tile_pool(name="ps_sum", bufs=2, space="PSUM") as smpool, \
                 tc.tile_pool(name="ps_av", bufs=2, space="PSUM") as avpool:
                pairs = [(qc, hp) for qc in range(QC) for hp in range(R // 2)]
                NP = len(pairs)
                pts = {}
                acc = {}
                Cp = mybir.ActivationFunctionType.Copy
                for s in range(NP * ST + 2):
                    if s < NP * ST:
                        j, tt = divmod(s, ST)
                        qc, hp = pairs[j]
                        qs = slice(qc * 512, (qc + 1) * 512)
                        hA, hB = 2 * hp, 2 * hp + 1
                        ks = KT[:, tt * 128 : (tt + 1) * 128]
                        psc = scpool.tile([128, 2, 512], F32, tag="pss",
                                          name=f"psc_{s}")
                        nc.tensor.matmul(psc[:, 0], ks, QT[:, hA, qs],
                                         start=True, stop=True)
                        nc.tensor.matmul(psc[:, 1], ks, QT[:, hB, qs],
                                         start=True, stop=True)
                        pt = ppool.tile([128, 2, 512], BF16, tag="pt",
                                        name=f"pt_{s}")
                        nc.scalar.activation(pt, psc, Exp)
                        pts[s] = pt
                    c = s - 2
                    if c >= 0:
                        jc, ttc = divmod(c, ST)
                        if ttc == 0:
                            acc[jc] = (
                                smpool.tile([128, 512], F32, tag="sums",
                                            name=f"sumA_{jc}"),
                                smpool.tile([128, 512], F32, tag="sums",
                                            name=f"sumB_{jc}"),
                                avpool.tile([128, 512], F32, tag="av",
                                            name=f"avA_{jc}"),
                                avpool.tile([128, 512], F32, tag="av",
                                            name=f"avB_{jc}"),
                            )
                        sums_A, sums_B, av_A, av_B = acc[jc]
                        ptc = pts.pop(c)
                        st_, sp_ = (ttc == 0), (ttc == ST - 1)
                        nc.tensor.matmul(sums_A, ones, ptc[:, 0],
                                         start=st_, stop=sp_)
                        nc.tensor.matmul(av_A, V[:, ttc], ptc[:, 0],
                                         start=st_, stop=sp_)
                        nc.tensor.matmul(sums_B, ones, ptc[:, 1],
                                         start=st_, stop=sp_)
                        nc.tensor.matmul(av_B, V[:, ttc], ptc[:, 1],
                                         start=st_, stop=sp_)
                        if ttc == ST - 1:
                            # drain + normalize for pair jc, off critical path
                            qc, hp = pairs[jc]
                            qs = slice(qc * 512, (qc + 1) * 512)
                            hA, hB = 2 * hp, 2 * hp + 1
                            last = jc == NP - 1
                            ssA = spool.tile([128, 512], F32, tag="ssb",
                                             name=f"ssA_{jc}")
                            avsA = avspool.tile([128, 512], BF16, tag="avsb",
                                                name=f"avsA_{jc}")
                            ssB = spool.tile([128, 512], F32, tag="ssb",
                                             name=f"ssB_{jc}")
                            avsB = avspool.tile([128, 512], BF16, tag="avsb",
                                                name=f"avsB_{jc}")
                            if last:
                                # split drains ACT/DVE so phase-3 PSUM banks
                                # free fast; recip+normalize deferred into
                                # phase 3 (pool-close must not wait on them)
                                nc.scalar.activation(ssA, sums_A, Cp)
                                nc.scalar.activation(avsA, av_A, Cp)
                                nc.vector.tensor_copy(ssB, sums_B)
                                nc.vector.tensor_copy(avsB, av_B)
                                deferred = (hA, hB, qs, ssA, avsA, ssB, avsB)
                            else:
                                nc.vector.tensor_copy(ssA, sums_A)
                                nc.vector.tensor_copy(avsA, av_A)
                                nc.vector.tensor_copy(ssB, sums_B)
                                nc.vector.tensor_copy(avsB, av_B)
                                rcA = rpool.tile([128, 512], F32, tag="rc",
                                                 name=f"rcA_{jc}")
                                nc.vector.reciprocal(rcA, ssA)
                                nc.vector.tensor_tensor(outT[:, hA, qs], avsA,
                                                        rcA, Mult)
                                rcB = rpool.tile([128, 512], F32, tag="rc",
                                                 name=f"rcB_{jc}")
                                nc.vector.reciprocal(rcB, ssB)
                                nc.vector.tensor_tensor(outT[:, hB, qs], avsB,
                                                        rcB, Mult)
                            del acc[jc]

            # ---- phase 3: output projection (transposed) ----
            with tc.tile_pool(name="ostage", bufs=8) as ostage, \
                 tc.tile_pool(name="ps_o", bufs=8, space="PSUM") as ps_o:
                for sc in range(SC):
                    if sc == 1:
                        # deferred last-pair normalize: 8us DVE chain runs
                        # here while sc==1 drains go to ACT / outs to sync
                        hA, hB, qs, ssA, avsA, ssB, avsB = deferred
                        rcA = rpool.tile([128, 512], F32, tag="rc")
                        nc.vector.reciprocal(rcA, ssA)
                        nc.vector.tensor_tensor(outT_last[:, 0], avsA, rcA,
                                                Mult)
                        rcB = rpool.tile([128, 512], F32, tag="rc")
                        nc.vector.reciprocal(rcB, ssB)
                        nc.vector.tensor_tensor(outT_last[:, 1], avsB, rcB,
                                                Mult)
                    for et in range(ET):
                        po = ps_o.tile([128, 512], F32, tag="po")
                        for h in range(R):
                            if sc == SC - 1 and h >= 2:
                                mv = outT_last[:, h - 2]
                            else:
                                mv = outT[:, h, sc * 512 : (sc + 1) * 512]
                            nc.tensor.matmul(
                                po,
                                wo_sb[:, h, et * 128 : (et + 1) * 128],
                                mv,
                                start=(h == 0), stop=(h == R - 1),
                            )
                        st = ostage.tile([128, 512], BF16, tag="ost")
                        if et % 2 == 0 and sc >= 2:
                            nc.vector.tensor_copy(st, po)
                        else:
                            nc.scalar.activation(
                                st, po, mybir.ActivationFunctionType.Copy)
                        eng = (nc.sync if (sc * ET + et) % 2 == 0 or sc <= 1
                               else nc.scalar)
                        eng.dma_start(
                            otd[et * 128 : (et + 1) * 128,
                                sc * 512 : (sc + 1) * 512],
                            st,
                        )

    _split_multi_waits(nc)
    return nc


def _prepare(x, Wq, bq, Wk, bk, Wv, bv, Wo, bo):
    """Host-side sharding: build per-core input maps (bf16)."""
    import ml_dtypes

    bf16 = ml_dtypes.bfloat16
    x = np.asarray(x, dtype=np.float32)
    Wq = np.asarray(Wq, dtype=np.float32)
    bq = np.asarray(bq, dtype=np.float32)
    Wk = np.asarray(Wk, dtype=np.float32)
    bk = np.asarray(bk, dtype=np.float32)
    Wv = np.asarray(Wv, dtype=np.float32)
    bv = np.asarray(bv, dtype=np.float32)
    Wo = np.asarray(Wo, dtype=np.float32)

    isd = np.float32(1.0 / np.sqrt(D))

    xTs = [np.ascontiguousarray(x[b].T).astype(bf16) for b in range(B)]
    wqs = [
        np.ascontiguousarray(Wq[:, g * R * D : (g + 1) * R * D] * isd).astype(bf16)
        for g in range(G)
    ]
    def _pmajor(wmat):
        return np.ascontiguousarray(
            wmat.reshape(ET, 128, -1).transpose(1, 0, 2)).astype(bf16)

    wks = [_pmajor(Wk[:, g * D : (g + 1) * D]) for g in range(G)]
    wvs = [_pmajor(Wv[:, g * D : (g + 1) * D]) for g in range(G)]
    wos = [np.ascontiguousarray(Wo[g * R * D : (g + 1) * R * D, :]).astype(bf16)
           for g in range(G)]
    in_maps = []
    for core in range(8):
        b, g = divmod(core, G)
        in_maps.append({
            "xT": xTs[b],
            "wq": wqs[g],
            "wkh": wks[g],
            "wvh": wvs[g],
            "wo": wos[g],
            "bqv": bq[g * R * D : (g + 1) * R * D] * isd,
            "bkv": bk[g * D : (g + 1) * D],
            "bvv": bv[g * D : (g + 1) * D],
        })
    return in_maps


def _gather(results, bo):
    bo = np.asarray(bo, dtype=np.float32)
    out = np.empty((B, S, E), dtype=np.float32)
    for b in range(B):
        acc = results[b * G]["ot"].astype(np.float32)
        for g in range(1, G):
            acc += results[b * G + g]["ot"].astype(np.float32)
        out[b] = acc.T + bo
    return out


def kernel(x, Wq, bq, Wk, bk, Wv, bv, Wo, bo):
    from concourse.bass_utils import run_bass_kernel_spmd

    if "nc" not in _cache:
        _cache["nc"] = _build_program()
    nc = _cache["nc"]
    in_maps = _prepare(x, Wq, bq, Wk, bk, Wv, bv, Wo, bo)
    res = run_bass_kernel_spmd(nc, in_maps, core_ids=list(range(8)))
    return _gather(res.results, bo)



# revision 10
# speedup vs baseline: 1.1334x; 1.1334x over previous
"""GQA attention kernel for 8 Trainium2 NeuronCores.

Sharding: core = (batch b, kv_group g), b in {0,1}, g in {0..3}.
Each core computes the 4 heads of one KV group for one batch and the
partial output projection for those heads; the host sums the 4 group
partials per batch.  Zero duplicated compute across cores.

All matmul operands are bf16 (fp32 PSUM accumulation).

v2 design — one merged PE stream (vs the v1 3-phase structure):
  The PE's pure-GEMM floor is ~1152 matmuls; v1 additionally spent 256
  matmuls (57us) on softmax denominators (ones-stationary partition
  reduction) and ran the output projection as a separate tail.  Here:
  - denominators: per attention step the exp'd P tile is accumulated on
    DVE (bf16 tensor_tensor add, 2x packed mode) into a per-pair acc;
    one ones-matmul per head per pair (16 total vs 256) reduces the
    partition dim.  1/sums = Exp(-Ln(sums)) on ACT: both functions live
    in the natural_log_exp_and_others table set (single load; the
    custom-DVE fast reciprocal does not compile on this walrus, and the
    stock InstReciprocal at ~6 cyc/elem would crowd the DVE).
  - the attention stream is ACT-bound at ~1.15us/step (exp of
    [128,2x512] at 1 elem/lane/cycle); the PE slack is filled by
    injecting Q-projection chains for the NEXT q-chunk (blocks 0-2) and
    output-projection chains (block 3) between the scores/AV matmuls.
  - stream order: K,V proj (all chunks) -> Q proj chunk 0 -> 128
    attention steps (8 head-pairs x 16 kv-tiles, consume lag 3) ->
    remaining Wo chains.  PSUM: scores rotation 4 banks + AV pair accum
    2 banks + shared proj/sums pool 2 banks = 8 exactly.
  - lag 3 (not 2) gives exp(s) ~4us before its consume deadline, so the
    ~2.6us boundary Ln/Exp burst on the ACT queue cannot stall the PE;
    the per-step acc-add is issued before the AV matmuls so the
    boundary ones-matmuls never wait on DVE.
  wk/wv host-prearranged partition-major; x/wq keep strided DMA
  patterns — an all-contiguous layout was measured (v1) to trigger a
  chip-wide ~20% DVFS clock drop.
"""

import numpy as np

# problem shape (hardcoded per contract)
B, S, E = 2, 2048, 2048
H, G, D = 16, 4, 128
R = H // G          # heads per kv group = 4
ST = S // 128       # 16 t-tiles
ET = E // 128       # 16 e-tiles
SC = S // 512       # 4 s-chunks
QC = S // 512       # 4 q-chunks

_cache = {}


def _split_multi_waits(nc, maxw=1):
    """Walrus in this container accepts only one sync-wait per
    instruction; move extra waits onto preceding same-engine NoOps."""
    from concourse import mybir

    n_split = 0
    for fn in nc.m.functions:
        for bb in fn.blocks:
            out = []
            changed = False
            for inst in bb.instructions:
                si = inst.sync_info
                waits = list(si.on_wait or []) if si is not None else []
                if len(waits) > maxw:
                    changed = True
                    n_split += 1
                    head, tail = waits[:-maxw], waits[-maxw:]
                    for j in range(0, len(head), maxw):
                        nop = mybir.InstNoOp(
                            name=f"{inst.name}-wsplit{j}", ins=[], outs=[]
                        )
                        nop.engine = inst.engine
                        nop.sync_info = mybir.SyncInfo(
                            on_wait=head[j : j + maxw], on_update=[]
                        )
                        out.append(nop)
                    si.on_wait = tail
                out.append(inst)
            if changed:
                bb.instructions = out
    return n_split


def _build_program():
    import contextlib

    import concourse.bass as bass
    import concourse.tile as tile
    from concourse import mybir

    BF16 = mybir.dt.bfloat16
    F32 = mybir.dt.float32
    Exp = mybir.ActivationFunctionType.Exp
    Ln = mybir.ActivationFunctionType.Ln
    Mult = mybir.AluOpType.mult
    Add = mybir.AluOpType.add

    nc = bass.Bass(target_bir_lowering=False)

    xT = nc.dram_tensor("xT", [E, S], BF16, kind="ExternalInput")
    wq = nc.dram_tensor("wq", [E, R * D], BF16, kind="ExternalInput")
    # wk/wv host-prearranged partition-major: their natural layout would
    # DMA as 256B rows at a fraction of peak
    wkh = nc.dram_tensor("wkh", [128, ET, D], BF16, kind="ExternalInput")
    wvh = nc.dram_tensor("wvh", [128, ET, D], BF16, kind="ExternalInput")
    wo = nc.dram_tensor("wo", [R * D, E], BF16, kind="ExternalInput")
    bqv = nc.dram_tensor("bqv", [R * D], F32, kind="ExternalInput")
    bkv = nc.dram_tensor("bkv", [D], F32, kind="ExternalInput")
    bvv = nc.dram_tensor("bvv", [D], F32, kind="ExternalInput")
    otd = nc.dram_tensor("ot", [E, S], BF16, kind="ExternalOutput")

    with tile.TileContext(nc) as tc:
        with contextlib.ExitStack() as ctx:
            consts = ctx.enter_context(tc.tile_pool(name="consts", bufs=1))
            big = ctx.enter_context(tc.tile_pool(name="big", bufs=1))

            bq_sb = consts.tile([128, R], F32)
            nc.gpsimd.dma_start(bq_sb, bqv.rearrange("(o p) -> p o", p=128))
            bk_sb = consts.tile([128, 1], F32)
            nc.gpsimd.dma_start(bk_sb, bkv.rearrange("(o p) -> p o", p=128))
            bv_sb = consts.tile([128, 1], F32)
            nc.gpsimd.dma_start(bv_sb, bvv.rearrange("(o p) -> p o", p=128))

            ones_f = consts.tile([128, 128], F32)
            nc.gpsimd.memset(ones_f, 1.0)
            ones = consts.tile([128, 128], BF16)
            nc.vector.tensor_copy(ones, ones_f)

            QT = big.tile([128, R, S], BF16)    # QT[d, h, q]
            KT = big.tile([128, S], BF16)       # KT[d, t]
            VT = big.tile([128, S], BF16)       # VT[d, t]
            V = big.tile([128, ST, D], BF16)    # V[t%128, tt, d]
            outT = big.tile([128, R, S], BF16)  # normalized attn out
            wo_sb = big.tile([128, R, E], BF16)
            wq_sb = big.tile([128, ET, R * D], BF16)
            wk_sb = big.tile([128, ET, D], BF16)
            wv_sb = big.tile([128, ET, D], BF16)
            # x chunks all resident (lifetimes overlap: chunk sc is read
            # again by the injected Q-proj of block sc-1)
            xts = [big.tile([128, ET, 512], BF16, name=f"xt{i}")
                   for i in range(SC)]

            # ---- input DMAs (weights: ACT queue; x: SP queue) ----
            nc.scalar.dma_start(wk_sb, wkh[:, :])
            nc.scalar.dma_start(wv_sb, wvh[:, :])

            def _wchunk(dst, src_t, e0, e1):
                nc.scalar.dma_start(
                    dst[:, e0:e1],
                    src_t[e0 * 128 : e1 * 128, :].rearrange(
                        "(o p) m -> p o m", p=128
                    ),
                )

            for e0, e1 in ((0, 4), (4, 8), (8, 12), (12, 16)):
                _wchunk(wq_sb, wq, e0, e1)
            for sc in range(SC):
                for k in range(4):
                    nc.sync.dma_start(
                        xts[sc][:, k * 4 : (k + 1) * 4],
                        xT[k * 512 : (k + 1) * 512,
                           sc * 512 : (sc + 1) * 512].rearrange(
                            "(o p) m -> p o m", p=128
                        ),
                    )

            ppool = ctx.enter_context(tc.tile_pool(name="probs", bufs=6))
            accpool = ctx.enter_context(tc.tile_pool(name="accs", bufs=2))
            upool = ctx.enter_context(tc.tile_pool(name="lns", bufs=2))
            rbpool = ctx.enter_context(tc.tile_pool(name="rbs", bufs=2))
            avspool = ctx.enter_context(tc.tile_pool(name="avsb", bufs=3))
            ostage = ctx.enter_context(tc.tile_pool(name="ostage", bufs=6))
            scpool = ctx.enter_context(
                tc.tile_pool(name="ps_sc", bufs=2, space="PSUM"))
            avpool = ctx.enter_context(
                tc.tile_pool(name="ps_av", bufs=1, space="PSUM"))
            # shared by projection chains AND per-pair sums matmuls;
            # scheduling keeps boundary steps free of chains
            projpool = ctx.enter_context(
                tc.tile_pool(name="ps_pj", bufs=2, space="PSUM"))

            # ---- solo phase: K,V all chunks, then Q chunk 0 ----
            for sc in range(SC):
                cs = slice(sc * 512, (sc + 1) * 512)
                pk = projpool.tile([128, 512], F32, tag="proj",
                                   name=f"pk_{sc}")
                pv = projpool.tile([128, 512], F32, tag="proj",
                                   name=f"pv_{sc}")
                # interleave K/V per 4-e-tile group: tracks x-piece DMAs
                for g4 in range(4):
                    for e in range(g4 * 4, g4 * 4 + 4):
                        nc.tensor.matmul(pk, wk_sb[:, e], xts[sc][:, e],
                                         start=(e == 0), stop=(e == ET - 1))
                    for e in range(g4 * 4, g4 * 4 + 4):
                        nc.tensor.matmul(pv, wv_sb[:, e], xts[sc][:, e],
                                         start=(e == 0), stop=(e == ET - 1))
                nc.scalar.add(KT[:, cs], pk, bk_sb[:, 0:1])
                nc.scalar.add(VT[:, cs], pv, bv_sb[:, 0:1])
                for tt in range(sc * 4, sc * 4 + 4):
                    nc.sync.dma_start_transpose(
                        V[:, tt], VT[:, tt * 128 : (tt + 1) * 128]
                    )

            for h in range(R):
                pq = projpool.tile([128, 512], F32, tag="proj",
                                   name=f"pq0_{h}")
                for e in range(ET):
                    nc.tensor.matmul(pq, wq_sb[:, e, h * 128 : (h + 1) * 128],
                                     xts[0][:, e],
                                     start=(e == 0), stop=(e == ET - 1))
                nc.scalar.add(QT[:, h, 0:512], pq, bq_sb[:, h : h + 1])

            # wo is needed from block 3 on; SP queue drains x by ~30us
            nc.sync.dma_start(wo_sb, wo.rearrange("(o p) m -> p o m", p=128))

            # ---- injected work: Q-proj chains (blocks 0-2), Wo chains ----
            out_dma_n = [0]

            def _emit_out_chunk(sc, et, eng):
                po = projpool.tile([128, 512], F32, tag="proj",
                                   name=f"po_{sc}_{et}")
                for h in range(R):
                    nc.tensor.matmul(
                        po, wo_sb[:, h, et * 128 : (et + 1) * 128],
                        outT[:, h, sc * 512 : (sc + 1) * 512],
                        start=(h == 0), stop=(h == R - 1),
                    )
                st = ostage.tile([128, 512], BF16, tag="ost")
                nc.vector.tensor_copy(st, po)
                eng.dma_start(
                    otd[et * 128 : (et + 1) * 128, sc * 512 : (sc + 1) * 512],
                    st,
                )

            # inject[local_step] -> list of thunks, per block
            def _qproj_sched(qn):
                """Q-projection for chunk qn: 4 chains of 16 matmuls in
                windows clear of the pair-boundary steps {1,2,17,18}."""
                sched = {}
                windows = [(4, 9), (10, 15), (21, 26), (27, 31)]
                counts6 = [3, 3, 3, 3, 3, 1]
                counts5 = [4, 3, 3, 3, 3]
                for h, (w0, w1) in enumerate(windows):
                    counts = counts6 if (w1 - w0) == 5 else counts5
                    pq = [None]

                    def _mk(h=h, qn=qn, pq=pq):
                        def _start():
                            pq[0] = projpool.tile([128, 512], F32, tag="proj",
                                                  name=f"pq{qn}_{h}")
                        return _start
                    e = [0]
                    start_fn = _mk()
                    for i, (ls, cnt) in enumerate(
                            zip(range(w0, w1 + 1), counts)):
                        def _mms(h=h, qn=qn, pq=pq, e=e, cnt=cnt,
                                 first=(i == 0), start_fn=start_fn):
                            if first:
                                start_fn()
                            for _ in range(cnt):
                                ei = e[0]
                                nc.tensor.matmul(
                                    pq[0],
                                    wq_sb[:, ei, h * 128 : (h + 1) * 128],
                                    xts[qn][:, ei],
                                    start=(ei == 0), stop=(ei == ET - 1),
                                )
                                e[0] += 1
                        sched.setdefault(ls, []).append(_mms)

                    def _drain(h=h, qn=qn, pq=pq):
                        nc.vector.tensor_scalar_add(
                            QT[:, h, qn * 512 : (qn + 1) * 512], pq[0],
                            bq_sb[:, h : h + 1])
                    sched.setdefault(w1, []).append(_drain)
                return sched

            def _wo_sched():
                """11 Wo chains for output chunk 0 inside block 3."""
                sched = {}
                slots = [(4, 5), (6, 7), (8, 9), (10, 11), (12, 13),
                         (14, 15), (21, 22), (23, 24), (25, 26), (27, 28),
                         (29, 30)]
                for et, (l0, l1) in enumerate(slots):
                    def _mk(et=et):
                        po_ref = [None]

                        def _first():
                            po_ref[0] = projpool.tile(
                                [128, 512], F32, tag="proj", name=f"po3_{et}")
                            for h in range(2):
                                nc.tensor.matmul(
                                    po_ref[0],
                                    wo_sb[:, h, et * 128 : (et + 1) * 128],
                                    outT[:, h, 0:512],
                                    start=(h == 0), stop=False,
                                )

                        def _second():
                            for h in range(2, R):
                                nc.tensor.matmul(
                                    po_ref[0],
                                    wo_sb[:, h, et * 128 : (et + 1) * 128],
                                    outT[:, h, 0:512],
                                    start=False, stop=(h == R - 1),
                                )
                            st = ostage.tile([128, 512], BF16, tag="ost")
                            nc.vector.tensor_copy(st, po_ref[0])
                            nc.sync.dma_start(
                                otd[et * 128 : (et + 1) * 128, 0:512], st)
                        return _first, _second
                    f1, f2 = _mk()
                    sched.setdefault(l0, []).append(f1)
                    sched.setdefault(l1, []).append(f2)
                return sched

            # ---- attention stream: 8 pairs x 16 t-tiles, lag-2 consume ----
            pairs = [(qc, hp) for qc in range(QC) for hp in range(R // 2)]
            NP = len(pairs)
            pts = {}
            av_cur = [None]
            acc_cur = [None]
            block_scheds = [_qproj_sched(1), _qproj_sched(2), _qproj_sched(3),
                            _wo_sched()]

            def _boundary(j):
                """End of pair j: denominators, reciprocal, normalize."""
                qc, hp = pairs[j]
                qs = slice(qc * 512, (qc + 1) * 512)
                hA, hB = 2 * hp, 2 * hp + 1
                acc = acc_cur[0]
                avp = av_cur[0]
                avs = avspool.tile([128, 2, 512], BF16, tag="avsb",
                                   name=f"avs_{j}")
                nc.vector.tensor_copy(avs, avp)  # frees av psum for next pair
                sA = projpool.tile([128, 512], F32, tag="proj",
                                   name=f"sA_{j}")
                nc.tensor.matmul(sA, ones, acc[:, 0], start=True, stop=True)
                sB = projpool.tile([128, 512], F32, tag="proj",
                                   name=f"sB_{j}")
                nc.tensor.matmul(sB, ones, acc[:, 1], start=True, stop=True)
                u = upool.tile([128, 2, 512], F32, tag="ln", name=f"u_{j}")
                nc.scalar.activation(u[:, 0], sA, Ln)
                nc.scalar.activation(u[:, 1], sB, Ln)
                r = rbpool.tile([128, 2, 512], BF16, tag="rb", name=f"r_{j}")
                nc.scalar.activation(r, u, Exp, scale=-1.0)
                nc.vector.tensor_tensor(outT[:, hA, qs], avs[:, 0], r[:, 0],
                                        Mult)
                nc.vector.tensor_tensor(outT[:, hB, qs], avs[:, 1], r[:, 1],
                                        Mult)

            for s in range(NP * ST + 3):
                c = s - 3
                # acc-add first: gives the pair-final add a step of lead
                # time so the boundary ones-matmuls never stall the PE
                if c >= 0:
                    jc, ttc = divmod(c, ST)
                    ptc = pts[c]
                    if ttc == 1:
                        acc_cur[0] = accpool.tile([128, 2, 512], BF16,
                                                  tag="acc", name=f"acc_{jc}")
                        nc.vector.tensor_tensor(acc_cur[0], pts[c - 1], ptc,
                                                Add)
                        del pts[c - 1]  # kept past its consume for this add
                    elif ttc > 1:
                        nc.vector.tensor_tensor(acc_cur[0], acc_cur[0], ptc,
                                                Add)
                if s < NP * ST:
                    j, tt = divmod(s, ST)
                    qc, hp = pairs[j]
                    qs = slice(qc * 512, (qc + 1) * 512)
                    hA, hB = 2 * hp, 2 * hp + 1
                    ks = KT[:, tt * 128 : (tt + 1) * 128]
                    psc = scpool.tile([128, 2, 512], F32, tag="pss",
                                      name=f"psc_{s}")
                    nc.tensor.matmul(psc[:, 0], ks, QT[:, hA, qs],
                                     start=True, stop=True)
                    nc.tensor.matmul(psc[:, 1], ks, QT[:, hB, qs],
                                     start=True, stop=True)
                    pt = ppool.tile([128, 2, 512], BF16, tag="pt",
                                    name=f"pt_{s}")
                    nc.scalar.activation(pt, psc, Exp)
                    pts[s] = pt
                    # injected projection work for this block-local step
                    for fn in block_scheds[s // 32].get(s % 32, ()):
                        fn()
                if c >= 0:
                    jc, ttc = divmod(c, ST)
                    # pt(c) with ttc==0 stays alive one more step: the
                    # ttc==1 acc-add reads it
                    ptc = pts[c] if ttc == 0 else pts.pop(c)
                    if ttc == 0:
                        av_cur[0] = avpool.tile([128, 2, 512], F32, tag="av",
                                                name=f"av_{jc}")
                    st_, sp_ = (ttc == 0), (ttc == ST - 1)
                    nc.tensor.matmul(av_cur[0][:, 0], V[:, ttc], ptc[:, 0],
                                     start=st_, stop=sp_)
                    nc.tensor.matmul(av_cur[0][:, 1], V[:, ttc], ptc[:, 1],
                                     start=st_, stop=sp_)
                    if ttc == ST - 1:
                        _boundary(jc)

            # ---- tail: remaining output-projection chains ----
            tail = [(0, et) for et in range(11, ET)]
            tail += [(sc, et) for sc in (1, 2, 3) for et in range(ET)]
            for i, (sc, et) in enumerate(tail):
                _emit_out_chunk(sc, et, nc.sync if i % 2 == 0 else nc.scalar)

    _split_multi_waits(nc)
    return nc


def _prepare(x, Wq, bq, Wk, bk, Wv, bv, Wo, bo):
    """Host-side sharding: build per-core input maps (bf16)."""
    import ml_dtypes

    bf16 = ml_dtypes.bfloat16
    x = np.asarray(x, dtype=np.float32)
    Wq = np.asarray(Wq, dtype=np.float32)
    bq = np.asarray(bq, dtype=np.float32)
    Wk = np.asarray(Wk, dtype=np.float32)
    bk = np.asarray(bk, dtype=np.float32)
    Wv = np.asarray(Wv, dtype=np.float32)
    bv = np.asarray(bv, dtype=np.float32)
    Wo = np.asarray(Wo, dtype=np.float32)

    isd = np.float32(1.0 / np.sqrt(D))

    xTs = [np.ascontiguousarray(x[b].T).astype(bf16) for b in range(B)]
    wqs = [
        np.ascontiguousarray(Wq[:, g * R * D : (g + 1) * R * D] * isd).astype(bf16)
        for g in range(G)
    ]
    def _pmajor(wmat):
        return np.ascontiguousarray(
            wmat.reshape(ET, 128, -1).transpose(1, 0, 2)).astype(bf16)

    wks = [_pmajor(Wk[:, g * D : (g + 1) * D]) for g in range(G)]
    wvs = [_pmajor(Wv[:, g * D : (g + 1) * D]) for g in range(G)]
    wos = [np.ascontiguousarray(Wo[g * R * D : (g + 1) * R * D, :]).astype(bf16)
           for g in range(G)]
    in_maps = []
    for core in range(8):
        b, g = divmod(core, G)
        in_maps.append({
            "xT": xTs[b],
            "wq": wqs[g],
            "wkh": wks[g],
            "wvh": wvs[g],
            "wo": wos[g],
            "bqv": bq[g * R * D : (g + 1) * R * D] * isd,
            "bkv": bk[g * D : (g + 1) * D],
            "bvv": bv[g * D : (g + 1) * D],
        })
    return in_maps


def _gather(results, bo):
    bo = np.asarray(bo, dtype=np.float32)
    out = np.empty((B, S, E), dtype=np.float32)
    for b in range(B):
        acc = results[b * G]["ot"].astype(np.float32)
        for g in range(1, G):
            acc += results[b * G + g]["ot"].astype(np.float32)
        out[b] = acc.T + bo
    return out


def kernel(x, Wq, bq, Wk, bk, Wv, bv, Wo, bo):
    from concourse.bass_utils import run_bass_kernel_spmd

    if "nc" not in _cache:
        _cache["nc"] = _build_program()
    nc = _cache["nc"]
    in_maps = _prepare(x, Wq, bq, Wk, bk, Wv, bv, Wo, bo)
    res = run_bass_kernel_spmd(nc, in_maps, core_ids=list(range(8)))
    return _gather(res.results, bo)


# revision 13
# speedup vs baseline: 1.1367x; 1.0029x over previous
"""GQA attention kernel for 8 Trainium2 NeuronCores.

Sharding: core = (batch b, kv_group g), b in {0,1}, g in {0..3}.
Each core computes the 4 heads of one KV group for one batch and the
partial output projection for those heads; the host sums the 4 group
partials per batch.  Zero duplicated compute across cores.

All matmul operands are bf16 (fp32 PSUM accumulation).

v2 design — one merged PE stream (vs the v1 3-phase structure):
  The PE's pure-GEMM floor is ~1152 matmuls; v1 additionally spent 256
  matmuls (57us) on softmax denominators (ones-stationary partition
  reduction) and ran the output projection as a separate tail.  Here:
  - denominators: per attention step the exp'd P tile is accumulated on
    DVE (bf16 tensor_tensor add, 2x packed mode) into a per-pair acc;
    one ones-matmul per head per pair (16 total vs 256) reduces the
    partition dim.  1/sums = Exp(-Ln(sums)) on ACT: both functions live
    in the natural_log_exp_and_others table set (single load; the
    custom-DVE fast reciprocal does not compile on this walrus, and the
    stock InstReciprocal at ~6 cyc/elem would crowd the DVE).
  - the attention stream is ACT-bound at ~1.15us/step (exp of
    [128,2x512] at 1 elem/lane/cycle); the PE slack is filled by
    injecting Q-projection chains for the NEXT q-chunk (blocks 0-2) and
    output-projection chains (block 3) between the scores/AV matmuls.
  - stream order: K,V proj (all chunks) -> Q proj chunk 0 -> 128
    attention steps (8 head-pairs x 16 kv-tiles, consume lag 3) ->
    remaining Wo chains.  PSUM: scores rotation 4 banks + AV pair accum
    2 banks + shared proj/sums pool 2 banks = 8 exactly.
  - lag 3 (not 2) gives exp(s) ~4us before its consume deadline, so the
    ~2.6us boundary Ln/Exp burst on the ACT queue cannot stall the PE;
    the per-step acc-add is issued before the AV matmuls so the
    boundary ones-matmuls never wait on DVE.
  wk/wv host-prearranged partition-major; x/wq keep strided DMA
  patterns — an all-contiguous layout was measured (v1) to trigger a
  chip-wide ~20% DVFS clock drop.
"""

import numpy as np

# problem shape (hardcoded per contract)
B, S, E = 2, 2048, 2048
H, G, D = 16, 4, 128
R = H // G          # heads per kv group = 4
ST = S // 128       # 16 t-tiles
ET = E // 128       # 16 e-tiles
SC = S // 512       # 4 s-chunks
QC = S // 512       # 4 q-chunks

_cache = {}


def _split_multi_waits(nc, maxw=1):
    """Walrus in this container accepts only one sync-wait per
    instruction; move extra waits onto preceding same-engine NoOps."""
    from concourse import mybir

    n_split = 0
    for fn in nc.m.functions:
        for bb in fn.blocks:
            out = []
            changed = False
            for inst in bb.instructions:
                si = inst.sync_info
                waits = list(si.on_wait or []) if si is not None else []
                if len(waits) > maxw:
                    changed = True
                    n_split += 1
                    head, tail = waits[:-maxw], waits[-maxw:]
                    for j in range(0, len(head), maxw):
                        nop = mybir.InstNoOp(
                            name=f"{inst.name}-wsplit{j}", ins=[], outs=[]
                        )
                        nop.engine = inst.engine
                        nop.sync_info = mybir.SyncInfo(
                            on_wait=head[j : j + maxw], on_update=[]
                        )
                        out.append(nop)
                    si.on_wait = tail
                out.append(inst)
            if changed:
                bb.instructions = out
    return n_split


def _build_program():
    import contextlib

    import concourse.bass as bass
    import concourse.tile as tile
    from concourse import mybir

    BF16 = mybir.dt.bfloat16
    F32 = mybir.dt.float32
    Exp = mybir.ActivationFunctionType.Exp
    Ln = mybir.ActivationFunctionType.Ln
    Mult = mybir.AluOpType.mult
    Add = mybir.AluOpType.add

    nc = bass.Bass(target_bir_lowering=False)

    xT = nc.dram_tensor("xT", [E, S], BF16, kind="ExternalInput")
    wq = nc.dram_tensor("wq", [E, R * D], BF16, kind="ExternalInput")
    # wk/wv host-prearranged partition-major: their natural layout would
    # DMA as 256B rows at a fraction of peak
    wkh = nc.dram_tensor("wkh", [128, ET, D], BF16, kind="ExternalInput")
    wvh = nc.dram_tensor("wvh", [128, ET, D], BF16, kind="ExternalInput")
    wo = nc.dram_tensor("wo", [R * D, E], BF16, kind="ExternalInput")
    bqv = nc.dram_tensor("bqv", [R * D], F32, kind="ExternalInput")
    bkv = nc.dram_tensor("bkv", [D], F32, kind="ExternalInput")
    bvv = nc.dram_tensor("bvv", [D], F32, kind="ExternalInput")
    otd = nc.dram_tensor("ot", [E, S], BF16, kind="ExternalOutput")

    with tile.TileContext(nc) as tc:
        with contextlib.ExitStack() as ctx:
            consts = ctx.enter_context(tc.tile_pool(name="consts", bufs=1))
            big = ctx.enter_context(tc.tile_pool(name="big", bufs=1))

            bq_sb = consts.tile([128, R], F32)
            nc.gpsimd.dma_start(bq_sb, bqv.rearrange("(o p) -> p o", p=128))
            bk_sb = consts.tile([128, 1], F32)
            nc.gpsimd.dma_start(bk_sb, bkv.rearrange("(o p) -> p o", p=128))
            bv_sb = consts.tile([128, 1], F32)
            nc.gpsimd.dma_start(bv_sb, bvv.rearrange("(o p) -> p o", p=128))

            ones_f = consts.tile([128, 128], F32)
            nc.gpsimd.memset(ones_f, 1.0)
            ones = consts.tile([128, 128], BF16)
            nc.vector.tensor_copy(ones, ones_f)

            QT = big.tile([128, R, S], BF16)    # QT[d, h, q]
            KT = big.tile([128, S], BF16)       # KT[d, t]
            VT = big.tile([128, S], BF16)       # VT[d, t]
            V = big.tile([128, ST, D], BF16)    # V[t%128, tt, d]
            outT = big.tile([128, R, S], BF16)  # normalized attn out
            wo_sb = big.tile([128, R, E], BF16)
            wq_sb = big.tile([128, ET, R * D], BF16)
            wk_sb = big.tile([128, ET, D], BF16)
            wv_sb = big.tile([128, ET, D], BF16)
            # x chunks all resident (lifetimes overlap: chunk sc is read
            # again by the injected Q-proj of block sc-1)
            xts = [big.tile([128, ET, 512], BF16, name=f"xt{i}")
                   for i in range(SC)]

            # ---- input DMAs (weights: ACT queue; x: SP queue) ----
            # Order by first-use so early transfers don't fight late ones
            # for the 16 HW DMA queues: wk -> x0 -> wv -> x1 -> wq -> x2,x3
            def _wchunk(dst, src_t, e0, e1):
                nc.scalar.dma_start(
                    dst[:, e0:e1],
                    src_t[e0 * 128 : e1 * 128, :].rearrange(
                        "(o p) m -> p o m", p=128
                    ),
                )

            def _xchunk(sc, k):
                nc.sync.dma_start(
                    xts[sc][:, k * 4 : (k + 1) * 4],
                    xT[k * 512 : (k + 1) * 512,
                       sc * 512 : (sc + 1) * 512].rearrange(
                        "(o p) m -> p o m", p=128
                    ),
                )

            nc.scalar.dma_start(wk_sb, wkh[:, :])
            for k in range(4):
                _xchunk(0, k)
            nc.scalar.dma_start(wv_sb, wvh[:, :])
            for k in range(4):
                _xchunk(1, k)
            for e0, e1 in ((0, 4), (4, 8), (8, 12), (12, 16)):
                _wchunk(wq_sb, wq, e0, e1)
            for sc in (2, 3):
                for k in range(4):
                    _xchunk(sc, k)

            ppool = ctx.enter_context(tc.tile_pool(name="probs", bufs=6))
            accpool = ctx.enter_context(tc.tile_pool(name="accs", bufs=2))
            upool = ctx.enter_context(tc.tile_pool(name="lns", bufs=2))
            rbpool = ctx.enter_context(tc.tile_pool(name="rbs", bufs=2))
            avspool = ctx.enter_context(tc.tile_pool(name="avsb", bufs=3))
            ostage = ctx.enter_context(tc.tile_pool(name="ostage", bufs=6))
            scpool = ctx.enter_context(
                tc.tile_pool(name="ps_sc", bufs=2, space="PSUM"))
            avpool = ctx.enter_context(
                tc.tile_pool(name="ps_av", bufs=1, space="PSUM"))
            # shared by projection chains AND per-pair sums matmuls;
            # scheduling keeps boundary steps free of chains
            projpool = ctx.enter_context(
                tc.tile_pool(name="ps_pj", bufs=2, space="PSUM"))

            # ---- solo phase: K,V all chunks, then Q chunk 0 ----
            for sc in range(SC):
                cs = slice(sc * 512, (sc + 1) * 512)
                pk = projpool.tile([128, 512], F32, tag="proj",
                                   name=f"pk_{sc}")
                pv = projpool.tile([128, 512], F32, tag="proj",
                                   name=f"pv_{sc}")
                # interleave K/V per 4-e-tile group: tracks x-piece DMAs
                for g4 in range(4):
                    for e in range(g4 * 4, g4 * 4 + 4):
                        nc.tensor.matmul(pk, wk_sb[:, e], xts[sc][:, e],
                                         start=(e == 0), stop=(e == ET - 1))
                    for e in range(g4 * 4, g4 * 4 + 4):
                        nc.tensor.matmul(pv, wv_sb[:, e], xts[sc][:, e],
                                         start=(e == 0), stop=(e == ET - 1))
                nc.scalar.add(KT[:, cs], pk, bk_sb[:, 0:1])
                nc.scalar.add(VT[:, cs], pv, bv_sb[:, 0:1])
                for tt in range(sc * 4, sc * 4 + 4):
                    nc.sync.dma_start_transpose(
                        V[:, tt], VT[:, tt * 128 : (tt + 1) * 128]
                    )

            for h in range(R):
                pq = projpool.tile([128, 512], F32, tag="proj",
                                   name=f"pq0_{h}")
                for e in range(ET):
                    nc.tensor.matmul(pq, wq_sb[:, e, h * 128 : (h + 1) * 128],
                                     xts[0][:, e],
                                     start=(e == 0), stop=(e == ET - 1))
                nc.scalar.add(QT[:, h, 0:512], pq, bq_sb[:, h : h + 1])

            # wo is needed from block 3 on; SP queue drains x by ~30us
            nc.sync.dma_start(wo_sb, wo.rearrange("(o p) m -> p o m", p=128))

            # ---- injected work: Q-proj chains (blocks 0-2), Wo chains ----
            out_dma_n = [0]

            def _emit_out_chunk(sc, et, eng):
                po = projpool.tile([128, 512], F32, tag="proj",
                                   name=f"po_{sc}_{et}")
                for h in range(R):
                    nc.tensor.matmul(
                        po, wo_sb[:, h, et * 128 : (et + 1) * 128],
                        outT[:, h, sc * 512 : (sc + 1) * 512],
                        start=(h == 0), stop=(h == R - 1),
                    )
                st = ostage.tile([128, 512], BF16, tag="ost")
                nc.vector.tensor_copy(st, po)
                eng.dma_start(
                    otd[et * 128 : (et + 1) * 128, sc * 512 : (sc + 1) * 512],
                    st,
                )

            # inject[local_step] -> list of thunks, per block
            def _qproj_sched(qn):
                """Q-projection for chunk qn: 4 chains of 16 matmuls in
                windows clear of the pair-boundary steps {1,2,17,18}."""
                sched = {}
                windows = [(4, 9), (10, 15), (21, 26), (27, 31)]
                counts6 = [3, 3, 3, 3, 3, 1]
                counts5 = [4, 3, 3, 3, 3]
                for h, (w0, w1) in enumerate(windows):
                    counts = counts6 if (w1 - w0) == 5 else counts5
                    pq = [None]

                    def _mk(h=h, qn=qn, pq=pq):
                        def _start():
                            pq[0] = projpool.tile([128, 512], F32, tag="proj",
                                                  name=f"pq{qn}_{h}")
                        return _start
                    e = [0]
                    start_fn = _mk()
                    for i, (ls, cnt) in enumerate(
                            zip(range(w0, w1 + 1), counts)):
                        def _mms(h=h, qn=qn, pq=pq, e=e, cnt=cnt,
                                 first=(i == 0), start_fn=start_fn):
                            if first:
                                start_fn()
                            for _ in range(cnt):
                                ei = e[0]
                                nc.tensor.matmul(
                                    pq[0],
                                    wq_sb[:, ei, h * 128 : (h + 1) * 128],
                                    xts[qn][:, ei],
                                    start=(ei == 0), stop=(ei == ET - 1),
                                )
                                e[0] += 1
                        sched.setdefault(ls, []).append(_mms)

                    def _drain(h=h, qn=qn, pq=pq):
                        nc.vector.tensor_scalar_add(
                            QT[:, h, qn * 512 : (qn + 1) * 512], pq[0],
                            bq_sb[:, h : h + 1])
                    sched.setdefault(w1, []).append(_drain)
                return sched

            def _wo_sched():
                """12 Wo chains for output chunk 0 inside block 3 (which
                is ACT-bound, so this PE work is free wall-time)."""
                sched = {}
                slots = [(4, 5), (6, 7), (8, 9), (10, 11), (12, 13),
                         (14, 15), (19, 20), (21, 22), (23, 24), (25, 26),
                         (27, 28), (29, 30)]
                for et, (l0, l1) in enumerate(slots):
                    def _mk(et=et):
                        po_ref = [None]

                        def _first():
                            po_ref[0] = projpool.tile(
                                [128, 512], F32, tag="proj", name=f"po3_{et}")
                            for h in range(2):
                                nc.tensor.matmul(
                                    po_ref[0],
                                    wo_sb[:, h, et * 128 : (et + 1) * 128],
                                    outT[:, h, 0:512],
                                    start=(h == 0), stop=False,
                                )

                        def _second():
                            for h in range(2, R):
                                nc.tensor.matmul(
                                    po_ref[0],
                                    wo_sb[:, h, et * 128 : (et + 1) * 128],
                                    outT[:, h, 0:512],
                                    start=False, stop=(h == R - 1),
                                )
                            st = ostage.tile([128, 512], BF16, tag="ost")
                            nc.vector.tensor_copy(st, po_ref[0])
                            nc.sync.dma_start(
                                otd[et * 128 : (et + 1) * 128, 0:512], st)
                        return _first, _second
                    f1, f2 = _mk()
                    sched.setdefault(l0, []).append(f1)
                    sched.setdefault(l1, []).append(f2)
                return sched

            # ---- attention stream: 8 pairs x 16 t-tiles, lag-2 consume ----
            pairs = [(qc, hp) for qc in range(QC) for hp in range(R // 2)]
            NP = len(pairs)
            pts = {}
            av_cur = [None]
            acc_cur = [None]
            block_scheds = [_qproj_sched(1), _qproj_sched(2), _qproj_sched(3),
                            _wo_sched()]

            def _boundary(j):
                """End of pair j: denominators, reciprocal, normalize."""
                qc, hp = pairs[j]
                qs = slice(qc * 512, (qc + 1) * 512)
                hA, hB = 2 * hp, 2 * hp + 1
                acc = acc_cur[0]
                avp = av_cur[0]
                avs = avspool.tile([128, 2, 512], BF16, tag="avsb",
                                   name=f"avs_{j}")
                nc.vector.tensor_copy(avs, avp)  # frees av psum for next pair
                sA = projpool.tile([128, 512], F32, tag="proj",
                                   name=f"sA_{j}")
                nc.tensor.matmul(sA, ones, acc[:, 0], start=True, stop=True)
                sB = projpool.tile([128, 512], F32, tag="proj",
                                   name=f"sB_{j}")
                nc.tensor.matmul(sB, ones, acc[:, 1], start=True, stop=True)
                u = upool.tile([128, 2, 512], F32, tag="ln", name=f"u_{j}")
                nc.scalar.activation(u[:, 0], sA, Ln)
                nc.scalar.activation(u[:, 1], sB, Ln)
                r = rbpool.tile([128, 2, 512], BF16, tag="rb", name=f"r_{j}")
                nc.scalar.activation(r, u, Exp, scale=-1.0)
                nc.vector.tensor_tensor(outT[:, hA, qs], avs[:, 0], r[:, 0],
                                        Mult)
                nc.vector.tensor_tensor(outT[:, hB, qs], avs[:, 1], r[:, 1],
                                        Mult)

            for s in range(NP * ST + 3):
                c = s - 3
                # acc-add first: gives the pair-final add a step of lead
                # time so the boundary ones-matmuls never stall the PE
                if c >= 0:
                    jc, ttc = divmod(c, ST)
                    ptc = pts[c]
                    if ttc == 1:
                        acc_cur[0] = accpool.tile([128, 2, 512], BF16,
                                                  tag="acc", name=f"acc_{jc}")
                        nc.vector.tensor_tensor(acc_cur[0], pts[c - 1], ptc,
                                                Add)
                        del pts[c - 1]  # kept past its consume for this add
                    elif ttc > 1:
                        nc.vector.tensor_tensor(acc_cur[0], acc_cur[0], ptc,
                                                Add)
                if s < NP * ST:
                    j, tt = divmod(s, ST)
                    qc, hp = pairs[j]
                    qs = slice(qc * 512, (qc + 1) * 512)
                    hA, hB = 2 * hp, 2 * hp + 1
                    ks = KT[:, tt * 128 : (tt + 1) * 128]
                    psc = scpool.tile([128, 2, 512], F32, tag="pss",
                                      name=f"psc_{s}")
                    nc.tensor.matmul(psc[:, 0], ks, QT[:, hA, qs],
                                     start=True, stop=True)
                    nc.tensor.matmul(psc[:, 1], ks, QT[:, hB, qs],
                                     start=True, stop=True)
                    pt = ppool.tile([128, 2, 512], BF16, tag="pt",
                                    name=f"pt_{s}")
                    nc.scalar.activation(pt, psc, Exp)
                    pts[s] = pt
                    # injected projection work for this block-local step
                    for fn in block_scheds[s // 32].get(s % 32, ()):
                        fn()
                if c >= 0:
                    jc, ttc = divmod(c, ST)
                    # pt(c) with ttc==0 stays alive one more step: the
                    # ttc==1 acc-add reads it
                    ptc = pts[c] if ttc == 0 else pts.pop(c)
                    if ttc == 0:
                        av_cur[0] = avpool.tile([128, 2, 512], F32, tag="av",
                                                name=f"av_{jc}")
                    st_, sp_ = (ttc == 0), (ttc == ST - 1)
                    nc.tensor.matmul(av_cur[0][:, 0], V[:, ttc], ptc[:, 0],
                                     start=st_, stop=sp_)
                    nc.tensor.matmul(av_cur[0][:, 1], V[:, ttc], ptc[:, 1],
                                     start=st_, stop=sp_)
                    if ttc == ST - 1:
                        _boundary(jc)

            # ---- tail: remaining output-projection chains ----
            tail = [(0, et) for et in range(12, ET)]
            tail += [(sc, et) for sc in (1, 2, 3) for et in range(ET)]
            qs3 = (nc.sync, nc.scalar, nc.gpsimd)
            for i, (sc, et) in enumerate(tail):
                _emit_out_chunk(sc, et, qs3[i % 3])

    _split_multi_waits(nc)
    return nc


def _prepare(x, Wq, bq, Wk, bk, Wv, bv, Wo, bo):
    """Host-side sharding: build per-core input maps (bf16)."""
    import ml_dtypes

    bf16 = ml_dtypes.bfloat16
    x = np.asarray(x, dtype=np.float32)
    Wq = np.asarray(Wq, dtype=np.float32)
    bq = np.asarray(bq, dtype=np.float32)
    Wk = np.asarray(Wk, dtype=np.float32)
    bk = np.asarray(bk, dtype=np.float32)
    Wv = np.asarray(Wv, dtype=np.float32)
    bv = np.asarray(bv, dtype=np.float32)
    Wo = np.asarray(Wo, dtype=np.float32)

    isd = np.float32(1.0 / np.sqrt(D))

    xTs = [np.ascontiguousarray(x[b].T).astype(bf16) for b in range(B)]
    wqs = [
        np.ascontiguousarray(Wq[:, g * R * D : (g + 1) * R * D] * isd).astype(bf16)
        for g in range(G)
    ]
    def _pmajor(wmat):
        return np.ascontiguousarray(
            wmat.reshape(ET, 128, -1).transpose(1, 0, 2)).astype(bf16)

    wks = [_pmajor(Wk[:, g * D : (g + 1) * D]) for g in range(G)]
    wvs = [_pmajor(Wv[:, g * D : (g + 1) * D]) for g in range(G)]
    wos = [np.ascontiguousarray(Wo[g * R * D : (g + 1) * R * D, :]).astype(bf16)
           for g in range(G)]
    in_maps = []
    for core in range(8):
        b, g = divmod(core, G)
        in_maps.append({
            "xT": xTs[b],
            "wq": wqs[g],
            "wkh": wks[g],
            "wvh": wvs[g],
            "wo": wos[g],
            "bqv": bq[g * R * D : (g + 1) * R * D] * isd,
            "bkv": bk[g * D : (g + 1) * D],
            "bvv": bv[g * D : (g + 1) * D],
        })
    return in_maps


def _gather(results, bo):
    bo = np.asarray(bo, dtype=np.float32)
    out = np.empty((B, S, E), dtype=np.float32)
    for b in range(B):
        acc = results[b * G]["ot"].astype(np.float32)
        for g in range(1, G):
            acc += results[b * G + g]["ot"].astype(np.float32)
        out[b] = acc.T + bo
    return out


def kernel(x, Wq, bq, Wk, bk, Wv, bv, Wo, bo):
    from concourse.bass_utils import run_bass_kernel_spmd

    if "nc" not in _cache:
        _cache["nc"] = _build_program()
    nc = _cache["nc"]
    in_maps = _prepare(x, Wq, bq, Wk, bk, Wv, bv, Wo, bo)
    res = run_bass_kernel_spmd(nc, in_maps, core_ids=list(range(8)))
    return _gather(res.results, bo)


# revision 22
# speedup vs baseline: 1.1465x; 1.0086x over previous
"""GQA attention kernel for 8 Trainium2 NeuronCores.

Sharding: core = (batch b, kv_group g), b in {0,1}, g in {0..3}.
Each core computes the 4 heads of one KV group for one batch and the
partial output projection for those heads; the host sums the 4 group
partials per batch.  Zero duplicated compute across cores.

All matmul operands are bf16 (fp32 PSUM accumulation).

v2 design — one merged PE stream (vs the v1 3-phase structure):
  The PE's pure-GEMM floor is ~1152 matmuls; v1 additionally spent 256
  matmuls (57us) on softmax denominators (ones-stationary partition
  reduction) and ran the output projection as a separate tail.  Here:
  - denominators: per attention step the exp'd P tile is accumulated on
    DVE (bf16 tensor_tensor add, 2x packed mode) into a per-pair acc;
    one ones-matmul per head per pair (16 total vs 256) reduces the
    partition dim.  1/sums = Exp(-Ln(sums)) on ACT: both functions live
    in the natural_log_exp_and_others table set (single load; the
    custom-DVE fast reciprocal does not compile on this walrus, and the
    stock InstReciprocal at ~6 cyc/elem would crowd the DVE).
  - the attention stream is ACT-bound at ~1.15us/step (exp of
    [128,2x512] at 1 elem/lane/cycle); the PE slack is filled by
    injecting Q-projection chains for the NEXT q-chunk (blocks 0-2) and
    output-projection chains (block 3) between the scores/AV matmuls.
  - stream order: K,V proj (all chunks) -> Q proj chunk 0 -> 128
    attention steps (8 head-pairs x 16 kv-tiles, consume lag 3) ->
    remaining Wo chains.  PSUM: scores rotation 4 banks + AV pair accum
    2 banks + shared proj/sums pool 2 banks = 8 exactly.
  - lag 3 (not 2) gives exp(s) ~4us before its consume deadline, so the
    ~2.6us boundary Ln/Exp burst on the ACT queue cannot stall the PE;
    the per-step acc-add is issued before the AV matmuls so the
    boundary ones-matmuls never wait on DVE.
  wk/wv host-prearranged partition-major; x/wq keep strided DMA
  patterns — an all-contiguous layout was measured (v1) to trigger a
  chip-wide ~20% DVFS clock drop.
"""

import numpy as np

# problem shape (hardcoded per contract)
B, S, E = 2, 2048, 2048
H, G, D = 16, 4, 128
R = H // G          # heads per kv group = 4
ST = S // 128       # 16 t-tiles
ET = E // 128       # 16 e-tiles
SC = S // 512       # 4 s-chunks
QC = S // 512       # 4 q-chunks

_cache = {}


def _split_multi_waits(nc, maxw=1):
    """Walrus in this container accepts only one sync-wait per
    instruction; move extra waits onto preceding same-engine NoOps."""
    from concourse import mybir

    n_split = 0
    for fn in nc.m.functions:
        for bb in fn.blocks:
            out = []
            changed = False
            for inst in bb.instructions:
                si = inst.sync_info
                waits = list(si.on_wait or []) if si is not None else []
                if len(waits) > maxw:
                    changed = True
                    n_split += 1
                    head, tail = waits[:-maxw], waits[-maxw:]
                    for j in range(0, len(head), maxw):
                        nop = mybir.InstNoOp(
                            name=f"{inst.name}-wsplit{j}", ins=[], outs=[]
                        )
                        nop.engine = inst.engine
                        nop.sync_info = mybir.SyncInfo(
                            on_wait=head[j : j + maxw], on_update=[]
                        )
                        out.append(nop)
                    si.on_wait = tail
                out.append(inst)
            if changed:
                bb.instructions = out
    return n_split


def _build_program():
    import contextlib

    import concourse.bass as bass
    import concourse.tile as tile
    from concourse import mybir

    BF16 = mybir.dt.bfloat16
    F32 = mybir.dt.float32
    Exp = mybir.ActivationFunctionType.Exp
    Ln = mybir.ActivationFunctionType.Ln
    Mult = mybir.AluOpType.mult
    Add = mybir.AluOpType.add

    nc = bass.Bass(target_bir_lowering=False)

    xT = nc.dram_tensor("xT", [E, S], BF16, kind="ExternalInput")
    wq = nc.dram_tensor("wq", [E, R * D], BF16, kind="ExternalInput")
    # wk/wv host-prearranged partition-major: their natural layout would
    # DMA as 256B rows at a fraction of peak
    wkh = nc.dram_tensor("wkh", [128, ET, D], BF16, kind="ExternalInput")
    wvh = nc.dram_tensor("wvh", [128, ET, D], BF16, kind="ExternalInput")
    wo = nc.dram_tensor("wo", [R * D, E], BF16, kind="ExternalInput")
    bqv = nc.dram_tensor("bqv", [R * D], F32, kind="ExternalInput")
    bkv = nc.dram_tensor("bkv", [D], F32, kind="ExternalInput")
    bvv = nc.dram_tensor("bvv", [D], F32, kind="ExternalInput")
    onesd = nc.dram_tensor("onesd", [128, 128], BF16, kind="ExternalInput")
    otd = nc.dram_tensor("ot", [E, S], BF16, kind="ExternalOutput")

    with tile.TileContext(nc) as tc:
        with contextlib.ExitStack() as ctx:
            consts = ctx.enter_context(tc.tile_pool(name="consts", bufs=1))
            big = ctx.enter_context(tc.tile_pool(name="big", bufs=1))

            bq_sb = consts.tile([128, R], F32)
            nc.gpsimd.dma_start(bq_sb, bqv.rearrange("(o p) -> p o", p=128))
            bk_sb = consts.tile([128, 1], F32)
            nc.gpsimd.dma_start(bk_sb, bkv.rearrange("(o p) -> p o", p=128))
            bv_sb = consts.tile([128, 1], F32)
            nc.gpsimd.dma_start(bv_sb, bvv.rearrange("(o p) -> p o", p=128))

            # host-provided (a gpsimd memset measured ~1us of preamble)
            ones = consts.tile([128, 128], BF16)
            nc.gpsimd.dma_start(ones, onesd[:, :])

            QT = big.tile([128, R, S], BF16)    # QT[d, h, q]
            KT = big.tile([128, S], BF16)       # KT[d, t]
            VT = big.tile([128, S], BF16)       # VT[d, t]
            V = big.tile([128, ST, D], BF16)    # V[t%128, tt, d]
            outT = big.tile([128, R, S], BF16)  # normalized attn out
            wo_sb = big.tile([128, R, E], BF16)
            wq_sb = big.tile([128, ET, R * D], BF16)
            wk_sb = big.tile([128, ET, D], BF16)
            wv_sb = big.tile([128, ET, D], BF16)
            # x chunks all resident (lifetimes overlap: chunk sc is read
            # again by the injected Q-proj of block sc-1)
            xts = [big.tile([128, ET, 512], BF16, name=f"xt{i}")
                   for i in range(SC)]

            # ---- input DMAs (weights: ACT queue; x: SP queue) ----
            # Order by first-use so early transfers don't fight late ones
            # for the 16 HW DMA queues: wk -> x0 -> wv -> x1 -> wq -> x2,x3
            def _wchunk(dst, src_t, e0, e1):
                nc.scalar.dma_start(
                    dst[:, e0:e1],
                    src_t[e0 * 128 : e1 * 128, :].rearrange(
                        "(o p) m -> p o m", p=128
                    ),
                )

            def _xchunk(sc, k):
                nc.sync.dma_start(
                    xts[sc][:, k * 4 : (k + 1) * 4],
                    xT[k * 512 : (k + 1) * 512,
                       sc * 512 : (sc + 1) * 512].rearrange(
                        "(o p) m -> p o m", p=128
                    ),
                )

            # wk/wv split and interleaved with x0 so the K/V chains of
            # chunk 0 can start as soon as the first pieces land
            for k in range(4):
                nc.scalar.dma_start(wk_sb[:, 4 * k : 4 * k + 4],
                                    wkh[:, 4 * k : 4 * k + 4])
                _xchunk(0, k)
                nc.scalar.dma_start(wv_sb[:, 4 * k : 4 * k + 4],
                                    wvh[:, 4 * k : 4 * k + 4])
            for k in range(4):
                _xchunk(1, k)
            for e0, e1 in ((0, 4), (4, 8), (8, 12), (12, 16)):
                _wchunk(wq_sb, wq, e0, e1)
            for sc in (2, 3):
                for k in range(4):
                    _xchunk(sc, k)

            ppool = ctx.enter_context(tc.tile_pool(name="probs", bufs=6))
            accpool = ctx.enter_context(tc.tile_pool(name="accs", bufs=2))
            upool = ctx.enter_context(tc.tile_pool(name="lns", bufs=2))
            rbpool = ctx.enter_context(tc.tile_pool(name="rbs", bufs=2))
            avspool = ctx.enter_context(tc.tile_pool(name="avsb", bufs=3))
            ostage = ctx.enter_context(tc.tile_pool(name="ostage", bufs=6))
            scpool = ctx.enter_context(
                tc.tile_pool(name="ps_sc", bufs=2, space="PSUM"))
            avpool = ctx.enter_context(
                tc.tile_pool(name="ps_av", bufs=1, space="PSUM"))
            # shared by projection chains AND per-pair sums matmuls;
            # scheduling keeps boundary steps free of chains
            projpool = ctx.enter_context(
                tc.tile_pool(name="ps_pj", bufs=2, space="PSUM"))

            # ---- solo phase: K,V all chunks, then Q chunk 0 ----
            for sc in range(SC):
                cs = slice(sc * 512, (sc + 1) * 512)
                pk = projpool.tile([128, 512], F32, tag="proj",
                                   name=f"pk_{sc}")
                pv = projpool.tile([128, 512], F32, tag="proj",
                                   name=f"pv_{sc}")
                # interleave K/V per 4-e-tile group: tracks x-piece DMAs
                for g4 in range(4):
                    for e in range(g4 * 4, g4 * 4 + 4):
                        nc.tensor.matmul(pk, wk_sb[:, e], xts[sc][:, e],
                                         start=(e == 0), stop=(e == ET - 1))
                    for e in range(g4 * 4, g4 * 4 + 4):
                        nc.tensor.matmul(pv, wv_sb[:, e], xts[sc][:, e],
                                         start=(e == 0), stop=(e == ET - 1))
                nc.scalar.add(KT[:, cs], pk, bk_sb[:, 0:1])
                nc.scalar.add(VT[:, cs], pv, bv_sb[:, 0:1])
                for tt in range(sc * 4, sc * 4 + 4):
                    nc.sync.dma_start_transpose(
                        V[:, tt], VT[:, tt * 128 : (tt + 1) * 128]
                    )

            for h in range(R):
                pq = projpool.tile([128, 512], F32, tag="proj",
                                   name=f"pq0_{h}")
                for e in range(ET):
                    nc.tensor.matmul(pq, wq_sb[:, e, h * 128 : (h + 1) * 128],
                                     xts[0][:, e],
                                     start=(e == 0), stop=(e == ET - 1))
                nc.scalar.add(QT[:, h, 0:512], pq, bq_sb[:, h : h + 1])

            # wo is needed from block 3 on; SP queue drains x by ~30us
            nc.sync.dma_start(wo_sb, wo.rearrange("(o p) m -> p o m", p=128))

            # ---- injected work: Q-proj chains (blocks 0-2), Wo chains ----
            out_dma_n = [0]

            def _emit_out_chunk(sc, et, eng):
                po = projpool.tile([128, 512], F32, tag="proj",
                                   name=f"po_{sc}_{et}")
                for h in range(R):
                    nc.tensor.matmul(
                        po, wo_sb[:, h, et * 128 : (et + 1) * 128],
                        outT[:, h, sc * 512 : (sc + 1) * 512],
                        start=(h == 0), stop=(h == R - 1),
                    )
                st = ostage.tile([128, 512], BF16, tag="ost")
                nc.vector.tensor_copy(st, po)
                eng.dma_start(
                    otd[et * 128 : (et + 1) * 128, sc * 512 : (sc + 1) * 512],
                    st,
                )

            # inject[local_step] -> list of thunks, per block
            def _qproj_sched(qn):
                """Q-projection for chunk qn: 4 chains of 16 matmuls in
                windows clear of the pair-boundary steps {1,2,17,18}."""
                sched = {}
                windows = [(6, 11), (12, 17), (22, 27), (28, 31)]
                counts6 = [3, 3, 3, 3, 3, 1]
                counts5 = [4, 4, 4, 4]
                for h, (w0, w1) in enumerate(windows):
                    counts = counts6 if (w1 - w0) == 5 else counts5
                    pq = [None]

                    def _mk(h=h, qn=qn, pq=pq):
                        def _start():
                            pq[0] = projpool.tile([128, 512], F32, tag="proj",
                                                  name=f"pq{qn}_{h}")
                        return _start
                    e = [0]
                    start_fn = _mk()
                    for i, (ls, cnt) in enumerate(
                            zip(range(w0, w1 + 1), counts)):
                        def _mms(h=h, qn=qn, pq=pq, e=e, cnt=cnt,
                                 first=(i == 0), start_fn=start_fn):
                            if first:
                                start_fn()
                            for _ in range(cnt):
                                ei = e[0]
                                nc.tensor.matmul(
                                    pq[0],
                                    wq_sb[:, ei, h * 128 : (h + 1) * 128],
                                    xts[qn][:, ei],
                                    start=(ei == 0), stop=(ei == ET - 1),
                                )
                                e[0] += 1
                        sched.setdefault(ls, []).append(_mms)

                    def _drain(h=h, qn=qn, pq=pq):
                        nc.vector.tensor_scalar_add(
                            QT[:, h, qn * 512 : (qn + 1) * 512], pq[0],
                            bq_sb[:, h : h + 1])
                    sched.setdefault(w1, []).append(_drain)
                return sched

            def _wo_sched():
                """11 Wo chains for output chunk 0 inside block 3."""
                sched = {}
                slots = [(5, 6), (7, 8), (9, 10), (11, 12), (13, 14),
                         (15, 16), (22, 23), (24, 25), (26, 27), (28, 29),
                         (30, 31)]
                for et, (l0, l1) in enumerate(slots):
                    def _mk(et=et):
                        po_ref = [None]

                        def _first():
                            po_ref[0] = projpool.tile(
                                [128, 512], F32, tag="proj", name=f"po3_{et}")
                            for h in range(2):
                                nc.tensor.matmul(
                                    po_ref[0],
                                    wo_sb[:, h, et * 128 : (et + 1) * 128],
                                    outT[:, h, 0:512],
                                    start=(h == 0), stop=False,
                                )

                        def _second():
                            for h in range(2, R):
                                nc.tensor.matmul(
                                    po_ref[0],
                                    wo_sb[:, h, et * 128 : (et + 1) * 128],
                                    outT[:, h, 0:512],
                                    start=False, stop=(h == R - 1),
                                )
                            st = ostage.tile([128, 512], BF16, tag="ost")
                            nc.vector.tensor_copy(st, po_ref[0])
                            nc.sync.dma_start(
                                otd[et * 128 : (et + 1) * 128, 0:512], st)
                        return _first, _second
                    f1, f2 = _mk()
                    sched.setdefault(l0, []).append(f1)
                    sched.setdefault(l1, []).append(f2)
                return sched

            # ---- attention stream: 8 pairs x 16 t-tiles, lag-2 consume ----
            pairs = [(qc, hp) for qc in range(QC) for hp in range(R // 2)]
            NP = len(pairs)
            pts = {}
            av_cur = [None]
            acc_cur = [None]
            block_scheds = [_qproj_sched(1), _qproj_sched(2), _qproj_sched(3),
                            _wo_sched()]

            deferred = {}

            def _boundary(j, s):
                """End of pair j: denominators, reciprocal, normalize.
                The three ACT ops are staggered 3 steps apart so each
                0.7-1.2us injection amortizes into the per-step ACT slack
                instead of delaying exp(s) past the psc-reuse deadline."""
                qc, hp = pairs[j]
                qs = slice(qc * 512, (qc + 1) * 512)
                hA, hB = 2 * hp, 2 * hp + 1
                acc = acc_cur[0]
                avp = av_cur[0]
                avs = avspool.tile([128, 2, 512], BF16, tag="avsb",
                                   name=f"avs_{j}")
                nc.vector.tensor_copy(avs, avp)  # frees av psum for next pair
                sA = projpool.tile([128, 512], F32, tag="proj",
                                   name=f"sA_{j}")
                nc.tensor.matmul(sA, ones, acc[:, 0], start=True, stop=True)
                sB = projpool.tile([128, 512], F32, tag="proj",
                                   name=f"sB_{j}")
                nc.tensor.matmul(sB, ones, acc[:, 1], start=True, stop=True)
                u = upool.tile([128, 2, 512], F32, tag="ln", name=f"u_{j}")
                r = rbpool.tile([128, 2, 512], BF16, tag="rb", name=f"r_{j}")

                def _ln_a():
                    nc.scalar.activation(u[:, 0], sA, Ln)

                def _ln_b():
                    nc.scalar.activation(u[:, 1], sB, Ln)

                def _finish():
                    nc.scalar.activation(r, u, Exp, scale=-1.0)
                    nc.vector.tensor_tensor(outT[:, hA, qs], avs[:, 0],
                                            r[:, 0], Mult)
                    nc.vector.tensor_tensor(outT[:, hB, qs], avs[:, 1],
                                            r[:, 1], Mult)

                if j == NP - 1:
                    _ln_a(); _ln_b(); _finish()
                else:
                    _ln_a()
                    deferred.setdefault(s + 3, []).append(_ln_b)
                    deferred.setdefault(s + 6, []).append(_finish)

            for s in range(NP * ST + 3):
                for fn in deferred.pop(s, ()):
                    fn()
                c = s - 3
                # acc-add first: gives the pair-final add a step of lead
                # time so the boundary ones-matmuls never stall the PE
                if c >= 0:
                    jc, ttc = divmod(c, ST)
                    ptc = pts[c]
                    if ttc == 1:
                        acc_cur[0] = accpool.tile([128, 2, 512], BF16,
                                                  tag="acc", name=f"acc_{jc}")
                        nc.vector.tensor_tensor(acc_cur[0], pts[c - 1], ptc,
                                                Add)
                        del pts[c - 1]  # kept past its consume for this add
                    elif ttc > 1:
                        nc.vector.tensor_tensor(acc_cur[0], acc_cur[0], ptc,
                                                Add)
                if s < NP * ST:
                    j, tt = divmod(s, ST)
                    qc, hp = pairs[j]
                    qs = slice(qc * 512, (qc + 1) * 512)
                    hA, hB = 2 * hp, 2 * hp + 1
                    ks = KT[:, tt * 128 : (tt + 1) * 128]
                    psc = scpool.tile([128, 2, 512], F32, tag="pss",
                                      name=f"psc_{s}")
                    nc.tensor.matmul(psc[:, 0], ks, QT[:, hA, qs],
                                     start=True, stop=True)
                    nc.tensor.matmul(psc[:, 1], ks, QT[:, hB, qs],
                                     start=True, stop=True)
                    pt = ppool.tile([128, 2, 512], BF16, tag="pt",
                                    name=f"pt_{s}")
                    nc.scalar.activation(pt, psc, Exp)
                    pts[s] = pt
                    # injected projection work for this block-local step
                    for fn in block_scheds[s // 32].get(s % 32, ()):
                        fn()
                if c >= 0:
                    jc, ttc = divmod(c, ST)
                    # pt(c) with ttc==0 stays alive one more step: the
                    # ttc==1 acc-add reads it
                    ptc = pts[c] if ttc == 0 else pts.pop(c)
                    if ttc == 0:
                        av_cur[0] = avpool.tile([128, 2, 512], F32, tag="av",
                                                name=f"av_{jc}")
                    st_, sp_ = (ttc == 0), (ttc == ST - 1)
                    nc.tensor.matmul(av_cur[0][:, 0], V[:, ttc], ptc[:, 0],
                                     start=st_, stop=sp_)
                    nc.tensor.matmul(av_cur[0][:, 1], V[:, ttc], ptc[:, 1],
                                     start=st_, stop=sp_)
                    if ttc == ST - 1:
                        _boundary(jc, s)

            # flush any boundary ops scheduled past the stream end
            for key in sorted(deferred):
                for fn in deferred.pop(key):
                    fn()

            # ---- tail: remaining output-projection chains ----
            tail = [(0, et) for et in range(11, ET)]
            tail += [(sc, et) for sc in (1, 2, 3) for et in range(ET)]
            qs3 = (nc.sync, nc.scalar, nc.gpsimd)
            for i, (sc, et) in enumerate(tail):
                _emit_out_chunk(sc, et, qs3[i % 3])

    _split_multi_waits(nc)
    return nc


def _prepare(x, Wq, bq, Wk, bk, Wv, bv, Wo, bo):
    """Host-side sharding: build per-core input maps (bf16)."""
    import ml_dtypes

    bf16 = ml_dtypes.bfloat16
    x = np.asarray(x, dtype=np.float32)
    Wq = np.asarray(Wq, dtype=np.float32)
    bq = np.asarray(bq, dtype=np.float32)
    Wk = np.asarray(Wk, dtype=np.float32)
    bk = np.asarray(bk, dtype=np.float32)
    Wv = np.asarray(Wv, dtype=np.float32)
    bv = np.asarray(bv, dtype=np.float32)
    Wo = np.asarray(Wo, dtype=np.float32)

    isd = np.float32(1.0 / np.sqrt(D))

    xTs = [np.ascontiguousarray(x[b].T).astype(bf16) for b in range(B)]
    wqs = [
        np.ascontiguousarray(Wq[:, g * R * D : (g + 1) * R * D] * isd).astype(bf16)
        for g in range(G)
    ]
    def _pmajor(wmat):
        return np.ascontiguousarray(
            wmat.reshape(ET, 128, -1).transpose(1, 0, 2)).astype(bf16)

    wks = [_pmajor(Wk[:, g * D : (g + 1) * D]) for g in range(G)]
    wvs = [_pmajor(Wv[:, g * D : (g + 1) * D]) for g in range(G)]
    wos = [np.ascontiguousarray(Wo[g * R * D : (g + 1) * R * D, :]).astype(bf16)
           for g in range(G)]
    ones128 = np.ones((128, 128), dtype=bf16)
    in_maps = []
    for core in range(8):
        b, g = divmod(core, G)
        in_maps.append({
            "xT": xTs[b],
            "wq": wqs[g],
            "wkh": wks[g],
            "wvh": wvs[g],
            "wo": wos[g],
            "bqv": bq[g * R * D : (g + 1) * R * D] * isd,
            "bkv": bk[g * D : (g + 1) * D],
            "bvv": bv[g * D : (g + 1) * D],
            "onesd": ones128,
        })
    return in_maps


def _gather(results, bo):
    bo = np.asarray(bo, dtype=np.float32)
    out = np.empty((B, S, E), dtype=np.float32)
    for b in range(B):
        acc = results[b * G]["ot"].astype(np.float32)
        for g in range(1, G):
            acc += results[b * G + g]["ot"].astype(np.float32)
        out[b] = acc.T + bo
    return out


def kernel(x, Wq, bq, Wk, bk, Wv, bv, Wo, bo):
    from concourse.bass_utils import run_bass_kernel_spmd

    if "nc" not in _cache:
        _cache["nc"] = _build_program()
    nc = _cache["nc"]
    in_maps = _prepare(x, Wq, bq, Wk, bk, Wv, bv, Wo, bo)
    res = run_bass_kernel_spmd(nc, in_maps, core_ids=list(range(8)))
    return _gather(res.results, bo)
